# revision 1
# baseline (speedup 1.0000x reference)
"""GCN (2x GCNConv + linear head) on 8 Trainium2 NeuronCores.

Strategy (graph/data parallel per the node-sharding hint):
  - dst nodes sharded across 8 cores (12500 each); 128x128 weights replicated.
  - Symmetric normalization folded into table rows: table[n] = dinv[n]*(x@W)[n];
    aggregate unweighted (self-loop = just another message), multiply by
    dinv[dst] afterwards.
  - Per layer: matmul the core's table shard (x passed feature-major so the
    chunk loads directly as matmul lhsT) -> AllGather the full [102400,128]
    bf16 message table (with a random graph every core needs nearly every
    node, so full replication beats point-to-point halos; split into 4
    quarter-wise AllGathers interleaved with the matmul chunks so gathers of
    chunk c start as soon as its collective lands) -> bulk gather of
    per-edge messages with `dma_gather` (<=1024 int16 indices per call, 4
    SWDGE queues round-robin) -> segmented reduction.
  - dma_gather has int16 indices, so the table is read in 4 chunks of 25600
    rows. Messages are aggregated in TWO levels: level 1 reduces each dst's
    messages *within a chunk* on a per-chunk degree-sorted grid (sorting
    keeps region widths uniform -> little padding) with a bf16 pairwise
    tree whose final level writes into a compact strip, DMA'd once per
    granule to DRAM partials; level 2 gathers each dst's 4 partials
    (uniform width -> trivially regular) in canonical dst order,
    accumulates in f32, and applies dinv/bias/relu.
  - Known HW pitfalls honored here: dma_gather must stay <=1024 indices per
    single_packet call; DVE tensor_tensor inputs must share a dtype (output
    upconversion bf16+bf16->f32 is fine, mixed inputs crash the device);
    indirect_dma_start consumes only one index per partition on HW.
"""

import numpy as np

import concourse.bacc as bacc
import concourse.bass as bass
import concourse.mybir as mybir
import concourse.tile as tile
from concourse import library_config
from concourse.bass_utils import run_bass_kernel_spmd
from concourse.masks import make_identity

N_NODES = 100000
N_CORES = 8
D = 128
P = 128

F32 = mybir.dt.float32
BF16 = mybir.dt.bfloat16
I16 = mybir.dt.int16
AX = mybir.AxisListType
ALU = mybir.AluOpType
ACTF = mybir.ActivationFunctionType

# message-table dtype: bf16 halves gather DMA bytes and doubles the DVE
# tree-add rate; level-2 accumulation and everything else stays f32
# (measured end-to-end absmax error ~2e-3 of output scale vs 3e-7 for f32).
TAB_DT = BF16

GMAX = 32          # max grid columns per message buffer granule
CALL_COLS = 8      # max grid columns per dma_gather call: single_packet
                   # SWDGE = 64 descriptors x 16 engines = 1024 indices
                   # (1536 was observed to DMA-abort on HW)


def _pack_idx(arr2d):
    """[cols, 128] int array (position i = col*128+p -> arr2d[col, p]) to the
    dma_gather idx tile layout: wrap by 16, replicate to 128 partitions."""
    flat = arr2d.reshape(-1)
    assert flat.size % 16 == 0
    wrapped = flat.reshape(-1, 16).T.astype(np.int16)  # [16, n/16]
    return np.tile(wrapped, (8, 1))  # [128, n/16]


def _granules(widths, gmax):
    """Split region widths into granules of consecutive regions with total
    width <= gmax. Returns (first_region, n_regions, col0, wsum)."""
    out = []
    r0 = 0
    R = len(widths)
    coloff = np.concatenate([[0], np.cumsum(widths)]).astype(int)
    while r0 < R:
        tot = int(widths[r0])
        r1 = r0 + 1
        while r1 < R and tot + int(widths[r1]) <= gmax:
            tot += int(widths[r1])
            r1 += 1
        out.append((r0, r1 - r0, int(coloff[r0]), tot))
        r0 = r1
    return out


def _plan(edge_index, n_nodes, n_cores):
    src = edge_index[0].astype(np.int64)
    dst = edge_index[1].astype(np.int64)
    E = src.shape[0]
    shard = n_nodes // n_cores
    # R regions, multiple of 4 so a table "chunk" = one quarter of every
    # core's shard = the output of one of 4 split AllGathers
    R = 4 * (-(-shard // (4 * P)))
    rows = R * P
    cap = rows // 4                     # local rows per quarter
    CH = 2 * rows                       # chunk rows = 8 cores * cap
    n_chunks = 4
    assert CH - 1 <= 32767, "chunk must be int16-addressable"

    base, rem = divmod(shard, 4)
    qreal = np.array([base + (1 if q < rem else 0) for q in range(4)])
    assert (qreal < cap).all(), "need a dummy (zero) row in every quarter"
    qstart = np.concatenate([[0], np.cumsum(qreal)])

    # node (core k, in-shard index il) -> local row l and table position
    il = np.arange(shard)
    q_of_il = np.searchsorted(qstart, il, "right") - 1
    local_of_il = q_of_il * cap + il - qstart[q_of_il]      # [shard]
    node_of_local = np.full(rows, -1, np.int64)
    node_of_local[local_of_il] = il
    core_of = np.arange(n_nodes) // shard
    loc = local_of_il[np.arange(n_nodes) - core_of * shard]
    pos = (loc // cap) * CH + core_of * cap + loc % cap

    deg = np.bincount(dst, minlength=n_nodes).astype(np.int64) + 1
    dinv = (1.0 / np.sqrt(deg.astype(np.float64))).astype(np.float32)

    # all messages: edges + self-loops, as (dst, src_table_pos)
    MD = np.concatenate([dst, np.arange(n_nodes)])
    MS = np.concatenate([pos[src], pos])
    MC = MS // CH                                     # chunk of each message
    NM = MD.shape[0]
    dloc = loc[MD - core_of[MD] * shard]              # dst local row
    dcore = core_of[MD]

    # per (core, local-row, chunk) counts
    cnt = np.zeros((n_cores, rows, n_chunks), np.int64)
    np.add.at(cnt, (dcore, dloc, MC), 1)

    # per-chunk degree-sorted grids
    rank = np.zeros((n_cores, rows, n_chunks), np.int64)
    W = np.zeros((n_chunks, R), np.int64)
    for c in range(n_chunks):
        for k in range(n_cores):
            order = np.argsort(-cnt[k, :, c], kind="stable")
            rank[k, order, c] = np.arange(rows)
        sorted_cnt = -np.sort(-cnt[:, :, c], axis=1)       # desc per core
        W[c] = np.maximum(sorted_cnt[:, ::P].max(axis=0), 1)
    assert W.max() <= GMAX, f"region width {W.max()} > GMAX"
    coloff = np.concatenate([np.zeros((n_chunks, 1), np.int64),
                             np.cumsum(W, axis=1)], axis=1)  # [n_chunks, R+1]
    chunk_cols = coloff[:, -1]
    grid_col0 = np.concatenate([[0], np.cumsum(chunk_cols)])
    tot_cols = int(grid_col0[-1])

    # fill grids [n_cores, tot_cols, 128] with chunk-local idx; pad points at
    # core 0's first dummy row of the chunk's quarter: chunk-local qreal[c]
    grid = np.empty((n_cores, tot_cols, P), np.int16)
    for c in range(n_chunks):
        grid[:, grid_col0[c]:grid_col0[c + 1]] = qreal[c]
    rk = rank[dcore, dloc, MC]
    r_m = rk // P
    p_m = rk % P
    # j-within-(dst,chunk)
    key = (dcore * rows + dloc) * n_chunks + MC
    morder = np.argsort(key, kind="stable")
    ks = key[morder]
    seg_first = np.ones(NM, bool)
    seg_first[1:] = ks[1:] != ks[:-1]
    seg_start_pos = np.where(seg_first)[0]
    seg_id = np.cumsum(seg_first) - 1
    j_sorted = np.arange(NM) - seg_start_pos[seg_id]
    j = np.empty(NM, np.int64)
    j[morder] = j_sorted
    col = grid_col0[MC] + coloff[MC, r_m] + j
    grid[dcore, col, p_m] = (MS - MC * CH).astype(np.int16)

    # level-1 partial row of (core, local, chunk): (c*R + r)*128 + p
    prow = (rank // P + np.arange(n_chunks)[None, None, :] * R) * P + rank % P

    # level-2 grids: half A = chunks 0,1 ; half B = chunks 2,3
    half_rows = 2 * R * P
    l2 = np.empty((n_cores, 2, 2 * R, P), np.int16)
    s_all = np.arange(rows)
    p2 = s_all % P
    r2 = s_all // P
    for h in range(2):
        for t in range(2):
            c = 2 * h + t
            v = prow[:, :, c] - h * half_rows
            assert v.min() >= 0 and v.max() < half_rows
            l2[:, h, 2 * r2 + t, p2] = v.astype(np.int16)

    # granule schedules (uniform across cores), interleaved so level-2
    # half A (needs chunks 0,1 only) overlaps level-1 of chunks 2,3
    call_meta = []
    off = 0

    def emit_l1(c):
        nonlocal off
        for (gr0, gnr, gc0, gw) in _granules(W[c], GMAX):
            regs = [(int(gr0 + i), int(coloff[c, gr0 + i] - gc0),
                     int(W[c, gr0 + i])) for i in range(gnr)]
            call_meta.append(("L1", c, gc0, gw, regs, off))
            off += 8 * gw

    def emit_l2(h):
        nonlocal off
        for (gr0, gnr, gc0, gw) in _granules([2] * R, GMAX):
            call_meta.append(("L2", h, gc0, gw, gr0, gnr, off))
            off += 8 * gw

    emit_l1(0)
    emit_l1(1)
    emit_l2(0)
    emit_l1(2)
    emit_l1(3)
    emit_l2(1)
    idx_cols = off

    # packed-idx layout follows the emission order above
    idx_all = np.empty((n_cores, P, idx_cols), np.int16)
    g01 = grid_col0[2]
    for k in range(n_cores):
        idx_all[k] = np.concatenate(
            [_pack_idx(grid[k, :g01]), _pack_idx(l2[k, 0]),
             _pack_idx(grid[k, g01:]), _pack_idx(l2[k, 1])],
            axis=1)

    dinv_rows = np.zeros((n_cores, P, R), np.float32)
    valid = node_of_local >= 0
    lv = np.where(valid)[0]
    for k in range(n_cores):
        dinv_rows[k, lv % P, lv // P] = dinv[k * shard + node_of_local[lv]]

    plan = dict(shard=shard, R=R, rows=rows, CH=CH, cap=cap,
                n_chunks=n_chunks, table_rows=rows * n_cores,
                half_rows=half_rows, call_meta=call_meta, idx_cols=idx_cols,
                tot_cols=tot_cols, node_of_local=node_of_local)
    return plan, idx_all, dinv_rows


def _build_program(plan, n_cores, repeat=1):
    R = plan["R"]
    rows = plan["rows"]
    CH = plan["CH"]
    table_rows = plan["table_rows"]
    half_rows = plan["half_rows"]
    call_meta = plan["call_meta"]
    idx_cols = plan["idx_cols"]

    cap = plan["cap"]
    nc = bacc.Bacc("TRN2", target_bir_lowering=False, debug=False,
                   enable_asserts=False, num_devices=n_cores,
                   num_swdge_queues=4)

    x_t = nc.dram_tensor("x_f", [D, rows], F32, kind="ExternalInput")
    w1_t = nc.dram_tensor("W1", [D, D], F32, kind="ExternalInput")
    w2_t = nc.dram_tensor("W2", [D, D], F32, kind="ExternalInput")
    wf_t = nc.dram_tensor("Wf", [1, D], F32, kind="ExternalInput")
    b1_t = nc.dram_tensor("b1", [1, D], F32, kind="ExternalInput")
    b2_t = nc.dram_tensor("b2", [1, D], F32, kind="ExternalInput")
    bf_t = nc.dram_tensor("bf", [1, 1], F32, kind="ExternalInput")
    idx_t = nc.dram_tensor("idx", [P, idx_cols], I16, kind="ExternalInput")
    dinv_t = nc.dram_tensor("dinv_rows", [P, R], F32, kind="ExternalInput")
    out_t = nc.dram_tensor("out", [P, R], F32, kind="ExternalOutput")

    rg = [list(range(n_cores))]

    with tile.TileContext(nc) as tc:
        with (
            tc.tile_pool(name="dram", bufs=1, space="DRAM") as dpool,
            tc.tile_pool(name="const", bufs=1) as cpool,
            tc.tile_pool(name="sb", bufs=2) as spool,
            tc.tile_pool(name="ps", bufs=2, space="PSUM") as ppool,
        ):
            nc.gpsimd.load_library(library_config.mlp)

            ag_in = [dpool.tile([rows, D], TAB_DT, name=f"ag_in{i}")
                     for i in range(2)]
            table = [dpool.tile([table_rows, D], TAB_DT, name=f"table{i}")
                     for i in range(2)]
            partials = [dpool.tile([2 * half_rows, D], TAB_DT,
                                   name=f"partials{i}")
                        for i in range(2)]

            # ---- constants ----
            ident = cpool.tile([P, P], F32)
            make_identity(nc, ident[:])
            w1s = cpool.tile([D, D], F32)
            nc.sync.dma_start(out=w1s[:], in_=w1_t.ap())
            w2s = cpool.tile([D, D], F32)
            nc.sync.dma_start(out=w2s[:], in_=w2_t.ap())
            wfs = cpool.tile([1, D], F32)
            nc.sync.dma_start(out=wfs[:], in_=wf_t.ap())
            b1s = cpool.tile([1, D], F32)
            nc.sync.dma_start(out=b1s[:], in_=b1_t.ap())
            b2s = cpool.tile([1, D], F32)
            nc.sync.dma_start(out=b2s[:], in_=b2_t.ap())
            bfs = cpool.tile([1, 1], F32)
            nc.sync.dma_start(out=bfs[:], in_=bf_t.ap())
            idx_s = cpool.tile([P, idx_cols], I16)
            nc.sync.dma_start(out=idx_s[:], in_=idx_t.ap())
            dinv_s = cpool.tile([P, R], F32)
            nc.sync.dma_start(out=dinv_s[:], in_=dinv_t.ap())

            ones1 = cpool.tile([1, P], F32)
            nc.vector.memset(ones1[:], 1.0)

            def bcast(vec_ap, n, nm):
                pb = ppool.tile([P, n], F32, tag="pbc")
                nc.tensor.matmul(pb[:], lhsT=ones1[:], rhs=vec_ap, start=True,
                                 stop=True)
                sb = cpool.tile([P, n], F32, name=f"bc_{nm}")
                nc.vector.tensor_copy(sb[:], pb[:])
                return sb

            b1b = bcast(b1s[:], D, "b1")
            b2b = bcast(b2s[:], D, "b2")
            wfb = bcast(wfs[:], D, "wf")
            bfb = bcast(bfs[:], 1, "bf")

            out_stage = cpool.tile([P, R], F32)
            relu1_t = cpool.tile([P, rows], F32)   # feature-major relu(layer1)

            qrr = [0]

            def next_q():
                qrr[0] = (qrr[0] + 1) % 4
                return qrr[0]

            for _rep in range(repeat):
                # ---- stage A: layer-1 table rows = dinv * (x @ W1) ----
                # x is provided feature-major so chunks load directly as lhsT
                chunks_per_quarter = cap // P
                for c in range(R):
                    xt = spool.tile([P, D], F32, tag="xt")
                    nc.sync.dma_start(out=xt[:],
                                      in_=x_t.ap()[:, c * P:(c + 1) * P])
                    pm = ppool.tile([P, D], F32, tag="psm")
                    nc.tensor.matmul(pm[:], lhsT=xt[:], rhs=w1s[:], start=True,
                                     stop=True)
                    h1c = spool.tile([P, D], TAB_DT, tag="h1c")
                    nc.vector.tensor_scalar_mul(h1c[:], pm[:], dinv_s[:, c:c + 1])
                    nc.sync.dma_start(out=ag_in[0][c * P:(c + 1) * P, :], in_=h1c[:])
                    if (c + 1) % chunks_per_quarter == 0:
                        qq = c // chunks_per_quarter
                        nc.gpsimd.collective_compute(
                            "AllGather", ALU.bypass, replica_groups=rg,
                            ins=[ag_in[0][qq * cap:(qq + 1) * cap, :]],
                            outs=[table[0][qq * CH:(qq + 1) * CH, :]])

                def aggregate(layer):
                    """gather + 2-level reduce from table[layer]; returns the
                    combined agg tile [P, R*D] (node-major, canonical order)."""
                    tab = table[layer]
                    par = partials[layer]
                    for meta in call_meta:
                        if meta[0] != "L1":
                            continue
                        (_, c, gc0, gw, regs, ioff) = meta
                        msg = spool.tile([P, GMAX * D], TAB_DT, tag="msg", bufs=2)
                        strip = spool.tile([P, GMAX * D], TAB_DT, tag="strip",
                                           bufs=2)
                        for a in range(0, gw, CALL_COLS):
                            b = min(a + CALL_COLS, gw)
                            nc.gpsimd.dma_gather(
                                msg[:, a * D:b * D].rearrange(
                                    "p (g f) -> p g f", f=D),
                                tab[c * CH:(c + 1) * CH, :],
                                idx_s[:, ioff + 8 * a:ioff + 8 * b],
                                (b - a) * P, (b - a) * P, D,
                                queue_num=next_q(),
                            )
                        gnr = len(regs)
                        for gi, (r, lo, w) in enumerate(regs):
                            # pairwise tree; final level lands in the strip
                            ww = w
                            while ww > 2:
                                h = (ww + 1) // 2
                                nadd = ww - h
                                nc.vector.tensor_tensor(
                                    out=msg[:, lo * D:(lo + nadd) * D],
                                    in0=msg[:, lo * D:(lo + nadd) * D],
                                    in1=msg[:, (lo + h) * D:(lo + h + nadd) * D],
                                    op=ALU.add)
                                ww = h
                            sl = strip[:, gi * D:(gi + 1) * D]
                            if ww == 2:
                                nc.vector.tensor_tensor(
                                    out=sl, in0=msg[:, lo * D:(lo + 1) * D],
                                    in1=msg[:, (lo + 1) * D:(lo + 2) * D],
                                    op=ALU.add)
                            else:
                                nc.vector.tensor_copy(
                                    sl, msg[:, lo * D:(lo + 1) * D])
                        base = (c * R + regs[0][0]) * P
                        nc.scalar.dma_start(
                            out=par[base:base + gnr * P, :].rearrange(
                                "(r p) f -> p r f", p=P),
                            in_=strip[:, :gnr * D].rearrange(
                                "p (r f) -> p r f", f=D))
                    # level 2: stream granules; agg[:, r*D:(r+1)*D] = sum of the
                    # dst's 4 partials (2 per half, columns interleaved per region)
                    agg = spool.tile([P, R * D], F32, tag="agg", bufs=1)
                    for h in range(2):
                        for meta in call_meta:
                            if meta[0] != "L2" or meta[1] != h:
                                continue
                            (_, _, gc0, gw, gr0, gnr, ioff) = meta
                            buf = spool.tile([P, GMAX * D], TAB_DT, tag="msg", bufs=2)
                            for a in range(0, gw, CALL_COLS):
                                b = min(a + CALL_COLS, gw)
                                nc.gpsimd.dma_gather(
                                    buf[:, a * D:b * D].rearrange(
                                        "p (g f) -> p g f", f=D),
                                    par[h * half_rows:(h + 1) * half_rows, :],
                                    idx_s[:, ioff + 8 * a:ioff + 8 * b],
                                    (b - a) * P, (b - a) * P, D,
                                    queue_num=next_q(),
                                )
                            v = buf[:, :gw * D].rearrange(
                                "p (r t f) -> p r t f", t=2, f=D)
                            aslice = agg[:, gr0 * D:(gr0 + gnr) * D].rearrange(
                                "p (r f) -> p r f", f=D)
                            if h == 0:
                                nc.vector.tensor_tensor(
                                    out=aslice, in0=v[:, :, 0, :],
                                    in1=v[:, :, 1, :], op=ALU.add)
                            else:
                                tmp = spool.tile([P, (GMAX // 2) * D], F32,
                                                 tag="l2tmp", bufs=2)
                                tslice = tmp[:, :gnr * D].rearrange(
                                    "p (r f) -> p r f", f=D)
                                nc.vector.tensor_tensor(
                                    out=tslice, in0=v[:, :, 0, :],
                                    in1=v[:, :, 1, :], op=ALU.add)
                                nc.vector.tensor_tensor(
                                    out=aslice, in0=aslice, in1=tslice,
                                    op=ALU.add)
                    return agg

                def scale_bias_relu(agg, bias_b):
                    nc.vector.tensor_tensor(
                        out=agg[:].rearrange("p (r f) -> p r f", f=D),
                        in0=agg[:].rearrange("p (r f) -> p r f", f=D),
                        in1=dinv_s[:, :, None].to_broadcast([P, R, D]),
                        op=ALU.mult)
                    nc.vector.tensor_tensor(
                        out=agg[:].rearrange("p (r f) -> p r f", f=D),
                        in0=agg[:].rearrange("p (r f) -> p r f", f=D),
                        in1=bias_b[:, None, :].to_broadcast([P, R, D]),
                        op=ALU.add)
                    nc.scalar.activation(agg[:], agg[:], ACTF.Relu)

                # ================= layer 1 =================
                agg1 = aggregate(0)
                scale_bias_relu(agg1, b1b)
                # transpose each 128-dst block into feature-major relu1_t
                for c in range(R):
                    pt2 = ppool.tile([P, D], F32, tag="pst")
                    nc.tensor.transpose(out=pt2[:], in_=agg1[:, c * D:(c + 1) * D],
                                        identity=ident[:])
                    nc.scalar.activation(relu1_t[:, c * P:(c + 1) * P], pt2[:],
                                         ACTF.Copy)
                # ---- stage D: layer-2 table rows = dinv * (relu1 @ W2) ----
                for c in range(R):
                    pm2 = ppool.tile([P, D], F32, tag="psm")
                    nc.tensor.matmul(pm2[:], lhsT=relu1_t[:, c * P:(c + 1) * P],
                                     rhs=w2s[:], start=True, stop=True)
                    h2c = spool.tile([P, D], TAB_DT, tag="h2c")
                    nc.vector.tensor_scalar_mul(h2c[:], pm2[:], dinv_s[:, c:c + 1])
                    nc.sync.dma_start(out=ag_in[1][c * P:(c + 1) * P, :], in_=h2c[:])
                    if (c + 1) % chunks_per_quarter == 0:
                        qq = c // chunks_per_quarter
                        nc.gpsimd.collective_compute(
                            "AllGather", ALU.bypass, replica_groups=rg,
                            ins=[ag_in[1][qq * cap:(qq + 1) * cap, :]],
                            outs=[table[1][qq * CH:(qq + 1) * CH, :]])

                # ================= layer 2 =================
                agg2 = aggregate(1)
                scale_bias_relu(agg2, b2b)

                # ---- final: out = relu2 @ Wf + bf ----
                for c in range(R):
                    fm = spool.tile([P, D], F32, tag="fm")
                    nc.vector.tensor_tensor(
                        out=fm[:], in0=agg2[:, c * D:(c + 1) * D], in1=wfb[:],
                        op=ALU.mult)
                    nc.vector.tensor_reduce(
                        out=out_stage[:, c:c + 1], in_=fm[:], axis=AX.X,
                        op=ALU.add)
                nc.vector.tensor_scalar_add(out_stage[:], out_stage[:], bfb[:, 0:1])
                nc.sync.dma_start(out=out_t.ap(), in_=out_stage[:])

    nc.compile()
    return nc


_CACHE: dict = {}
_PLAN_CACHE: dict = {}


def _plan_cached(edge_index, n_nodes, n_cores):
    import hashlib
    h = hashlib.sha256(np.ascontiguousarray(edge_index).tobytes()).hexdigest()
    key = (h, n_nodes, n_cores)
    if key not in _PLAN_CACHE:
        _PLAN_CACHE[key] = _plan(edge_index, n_nodes, n_cores)
    return _PLAN_CACHE[key]


def _prepare(x, W1, b1, W2, b2, Wf, bf, edge_index, n_nodes, n_cores):
    plan, idx_all, dinv_rows = _plan_cached(edge_index, n_nodes, n_cores)
    shard = plan["shard"]
    rows = plan["rows"]

    x = np.asarray(x, np.float32).reshape(n_cores, shard, D)
    nol = plan["node_of_local"]
    valid = nol >= 0
    x_in = np.zeros((n_cores, rows, D), np.float32)
    x_in[:, valid] = x[:, nol[valid]]

    common = {
        "W1": np.asarray(W1, np.float32).reshape(D, D),
        "W2": np.asarray(W2, np.float32).reshape(D, D),
        "Wf": np.asarray(Wf, np.float32).reshape(1, D),
        "b1": np.asarray(b1, np.float32).reshape(1, D),
        "b2": np.asarray(b2, np.float32).reshape(1, D),
        "bf": np.asarray(bf, np.float32).reshape(1, 1),
    }
    in_maps = []
    for k in range(n_cores):
        m = dict(common)
        m["x_f"] = np.ascontiguousarray(x_in[k].T)
        m["idx"] = np.ascontiguousarray(idx_all[k])
        m["dinv_rows"] = np.ascontiguousarray(dinv_rows[k])
        in_maps.append(m)
    return plan, in_maps


def _collect(results, plan, n_nodes, n_cores):
    shard = plan["shard"]
    nol = plan["node_of_local"]
    valid = np.where(nol >= 0)[0]
    out = np.empty(n_nodes, np.float32)
    for k in range(n_cores):
        vals = results[k]["out"]            # [P, R]
        out[k * shard + nol[valid]] = vals[valid % P, valid // P]
    return out


def kernel(x, W1, b1, W2, b2, Wf, bf, edge_index, _trace=False, _repeat=1):
    plan, in_maps = _prepare(x, W1, b1, W2, b2, Wf, bf, edge_index,
                             N_NODES, N_CORES)
    key = (_repeat, plan["idx_cols"], tuple(m[0] for m in plan["call_meta"]),
           tuple(m[3] for m in plan["call_meta"]))
    if key not in _CACHE:
        _CACHE[key] = _build_program(plan, N_CORES, repeat=_repeat)
    nc = _CACHE[key]
    res = run_bass_kernel_spmd(
        nc, in_maps, core_ids=list(range(N_CORES)), trace=_trace)
    out = _collect(res.results, plan, N_NODES, N_CORES)
    kernel.last_exec_time_ns = res.exec_time_ns
    kernel.last_results = res
    return out


kernel.last_exec_time_ns = None
kernel.last_results = None



# revision 17
# speedup vs baseline: 3.8944x; 3.8944x over previous
"""GCN (2x GCNConv + linear head) on 8 Trainium2 NeuronCores.

Strategy (graph/data parallel per the node-sharding hint):
  - dst nodes sharded across 8 cores (12500 each); 128x128 weights replicated.
  - Symmetric normalization folded into table rows: table[n] = dinv[n]*(x@W)[n];
    aggregate unweighted (self-loop = just another message), multiply by
    dinv[dst] afterwards.
  - Per layer: matmul the core's table shard (x passed feature-major so chunks
    load directly as matmul lhsT; 4 node-blocks batched per PSUM tile) ->
    quarter-wise AllGather of the [102400,128] bf16 table -> bulk gather of
    per-edge messages with `dma_gather` (<=1024 int16 indices per call) ->
    LEVEL-STRIP segmented reduction: within each table chunk, dsts are
    degree-sorted; the grid stores each dst's j-th message in "strip" j
    (one contiguous block range per level), so the whole reduction is one
    strided tensor_tensor add per level pair (pairwise tree over strips)
    instead of per-region trees. Level-1 partials land in DRAM in the same
    (c*R + r)*128 + p row layout; level 2 regathers each dst's 4 partials in
    canonical order (two int16-addressable halves) and combines.
  - This backend charges a large fixed cost per *instruction*, so everything
    is batched into as few, as large instructions as possible; bf16
    throughout the aggregation (output rel-err ~5e-3, gate 2e-2).
  - Known HW pitfalls honored: dma_gather <=1024 idx/call; DVE tensor_tensor
    input dtypes must match (bf16+bf16->f32 output upconvert OK); no
    fine-grained transposing DMAs (PE transposes instead).
"""

import numpy as np

import concourse.bacc as bacc
import concourse.bass as bass
import concourse.mybir as mybir
import concourse.tile as tile
from concourse import library_config
from concourse.bass_utils import run_bass_kernel_spmd
from concourse.masks import make_identity

N_NODES = 100000
N_CORES = 8
D = 128
P = 128

F32 = mybir.dt.float32
BF16 = mybir.dt.bfloat16
I16 = mybir.dt.int16
AX = mybir.AxisListType
ALU = mybir.AluOpType
ACTF = mybir.ActivationFunctionType

TAB_DT = BF16

SEG_BLOCKS = 176   # max grid columns (128-msg blocks) per L1 segment buffer
GMAX = 100         # max grid columns per L2 granule
CALL_COLS = 8      # grid columns per dma_gather call (8*128 = 1024 indices)


def _pack_idx(arr2d):
    """[cols, 128] int array (position i = col*128+p -> arr2d[col, p]) to the
    dma_gather idx tile layout: wrap by 16, replicate to 128 partitions."""
    flat = arr2d.reshape(-1)
    assert flat.size % 16 == 0
    wrapped = flat.reshape(-1, 16).T.astype(np.int16)  # [16, n/16]
    return np.tile(wrapped, (8, 1))  # [128, n/16]


def _granules(widths, gmax):
    out = []
    r0 = 0
    R = len(widths)
    coloff = np.concatenate([[0], np.cumsum(widths)]).astype(int)
    while r0 < R:
        tot = int(widths[r0])
        r1 = r0 + 1
        while r1 < R and tot + int(widths[r1]) <= gmax:
            tot += int(widths[r1])
            r1 += 1
        out.append((r0, r1 - r0, int(coloff[r0]), tot))
        r0 = r1
    return out


def _plan(edge_index, n_nodes, n_cores):
    src = edge_index[0].astype(np.int64)
    dst = edge_index[1].astype(np.int64)
    shard = n_nodes // n_cores
    R = 4 * (-(-shard // (4 * P)))
    rows = R * P
    cap = rows // 4                     # local rows per quarter
    CH = 2 * rows                       # chunk rows = 8 cores * cap
    n_chunks = 4
    assert CH - 1 <= 32767

    base, rem = divmod(shard, 4)
    qreal = np.array([base + (1 if q < rem else 0) for q in range(4)])
    assert (qreal < cap).all()
    qstart = np.concatenate([[0], np.cumsum(qreal)])

    il = np.arange(shard)
    q_of_il = np.searchsorted(qstart, il, "right") - 1
    local_of_il = q_of_il * cap + il - qstart[q_of_il]
    node_of_local = np.full(rows, -1, np.int64)
    node_of_local[local_of_il] = il
    core_of = np.arange(n_nodes) // shard
    loc = local_of_il[np.arange(n_nodes) - core_of * shard]
    pos = (loc // cap) * CH + core_of * cap + loc % cap

    deg = np.bincount(dst, minlength=n_nodes).astype(np.int64) + 1
    dinv = (1.0 / np.sqrt(deg.astype(np.float64))).astype(np.float32)

    MD = np.concatenate([dst, np.arange(n_nodes)])
    MS = np.concatenate([pos[src], pos])
    MC = MS // CH
    NM = MD.shape[0]
    dloc = loc[MD - core_of[MD] * shard]
    dcore = core_of[MD]

    cnt = np.zeros((n_cores, rows, n_chunks), np.int64)
    np.add.at(cnt, (dcore, dloc, MC), 1)

    # per-chunk degree-sorted ranks; region widths shared across cores
    rank = np.zeros((n_cores, rows, n_chunks), np.int64)
    W = np.zeros((n_chunks, R), np.int64)
    for c in range(n_chunks):
        for k in range(n_cores):
            order = np.argsort(-cnt[k, :, c], kind="stable")
            rank[k, order, c] = np.arange(rows)
        sorted_cnt = -np.sort(-cnt[:, :, c], axis=1)
        W[c] = np.maximum(sorted_cnt[:, ::P].max(axis=0), 1)
    assert W.max() <= SEG_BLOCKS

    # ---- L1 level-strip segments (layout shared across cores) ----
    maxW = int(W.max())
    col_of = np.full((n_chunks, R, maxW), -1, np.int64)
    seg_meta = [[] for _ in range(n_chunks)]
    chunk_cols = np.zeros(n_chunks + 1, np.int64)
    for c in range(n_chunks):
        Wc = W[c]
        cum = np.concatenate([[0], np.cumsum(Wc)])
        r0 = 0
        ccol = 0
        while r0 < R:
            r1 = r0 + 1
            while r1 < R and cum[r1 + 1] - cum[r0] <= SEG_BLOCKS:
                r1 += 1
            L = int(Wc[r0])
            posj = np.array([(Wc > j).sum() for j in range(L)])
            Rj = np.clip(np.minimum(posj, r1) - r0, 0, r1 - r0).astype(int)
            offj = np.concatenate([[0], np.cumsum(Rj)]).astype(int)
            segcols = int(offj[-1])
            assert segcols == int(cum[r1] - cum[r0])
            for j in range(L):
                rr = np.arange(r0, r0 + Rj[j])
                col_of[c, rr, j] = ccol + offj[j] + (rr - r0)
            ops = []
            offs = list(offj[:L])
            lens = list(Rj)
            Lv = L
            while Lv > 1:
                h = (Lv + 1) // 2
                for i in range(Lv - h):
                    ops.append((offs[i], offs[i + h], lens[i + h]))
                Lv = h
            seg_meta[c].append((r0, r1 - r0, segcols, ccol, tuple(ops)))
            ccol += segcols
            r0 = r1
        chunk_cols[c + 1] = ccol
    grid_col0 = np.concatenate([[0], np.cumsum(chunk_cols[1:])])
    tot_cols = int(grid_col0[-1])

    # fill grids [n_cores, tot_cols, 128]; pad -> zero row qreal[c]
    grid = np.empty((n_cores, tot_cols, P), np.int16)
    for c in range(n_chunks):
        grid[:, grid_col0[c]:grid_col0[c + 1]] = qreal[c]
    rk = rank[dcore, dloc, MC]
    r_m = rk // P
    p_m = rk % P
    key = (dcore * rows + dloc) * n_chunks + MC
    morder = np.argsort(key, kind="stable")
    ks = key[morder]
    seg_first = np.ones(NM, bool)
    seg_first[1:] = ks[1:] != ks[:-1]
    seg_start_pos = np.where(seg_first)[0]
    seg_id = np.cumsum(seg_first) - 1
    j_sorted = np.arange(NM) - seg_start_pos[seg_id]
    j = np.empty(NM, np.int64)
    j[morder] = j_sorted
    col = grid_col0[MC] + col_of[MC, r_m, j]
    assert (col_of[MC, r_m, j] >= 0).all()
    grid[dcore, col, p_m] = (MS - MC * CH).astype(np.int16)

    # level-1 partial row of (core, local, chunk): (c*R + r)*128 + p
    prow = (rank // P + np.arange(n_chunks)[None, None, :] * R) * P + rank % P

    half_rows = 2 * R * P
    l2 = np.empty((n_cores, 2, 2 * R, P), np.int16)
    s_all = np.arange(rows)
    p2 = s_all % P
    r2 = s_all // P
    for h in range(2):
        for t in range(2):
            c = 2 * h + t
            v = prow[:, :, c] - h * half_rows
            assert v.min() >= 0 and v.max() < half_rows
            l2[:, h, 2 * r2 + t, p2] = v.astype(np.int16)

    call_meta = []
    off = 0

    def emit_l1(c):
        nonlocal off
        for (r0, nr0, segcols, ccol, ops) in seg_meta[c]:
            call_meta.append(("L1", c, segcols, ops, r0, nr0, off))
            off += 8 * segcols

    def emit_l2(h):
        nonlocal off
        for (gr0, gnr, gc0, gw) in _granules([2] * R, GMAX):
            call_meta.append(("L2", h, gc0, gw, gr0, gnr, off))
            off += 8 * gw

    emit_l1(0)
    emit_l1(1)
    emit_l2(0)
    emit_l1(2)
    emit_l1(3)
    emit_l2(1)
    idx_cols = off

    idx_all = np.empty((n_cores, P, idx_cols), np.int16)
    g01 = grid_col0[2]
    for k in range(n_cores):
        idx_all[k] = np.concatenate(
            [_pack_idx(grid[k, :g01]), _pack_idx(l2[k, 0]),
             _pack_idx(grid[k, g01:]), _pack_idx(l2[k, 1])],
            axis=1)

    dinv_rows = np.zeros((n_cores, P, R), np.float32)
    valid = node_of_local >= 0
    lv = np.where(valid)[0]
    for k in range(n_cores):
        dinv_rows[k, lv % P, lv // P] = dinv[k * shard + node_of_local[lv]]

    plan = dict(shard=shard, R=R, rows=rows, CH=CH, cap=cap,
                n_chunks=n_chunks, table_rows=rows * n_cores,
                half_rows=half_rows, call_meta=call_meta, idx_cols=idx_cols,
                tot_cols=tot_cols, node_of_local=node_of_local)
    return plan, idx_all, dinv_rows


def _build_program(plan, n_cores, repeat=1, skip=frozenset()):
    R = plan["R"]
    rows = plan["rows"]
    CH = plan["CH"]
    table_rows = plan["table_rows"]
    half_rows = plan["half_rows"]
    call_meta = plan["call_meta"]
    idx_cols = plan["idx_cols"]
    cap = plan["cap"]

    nc = bacc.Bacc("TRN2", target_bir_lowering=False, debug=False,
                   enable_asserts=False, num_devices=n_cores,
                   num_swdge_queues=4)

    x_t = nc.dram_tensor("x_f", [D, rows], F32, kind="ExternalInput")
    w1_t = nc.dram_tensor("W1", [D, D], F32, kind="ExternalInput")
    w2_t = nc.dram_tensor("W2", [D, D], F32, kind="ExternalInput")
    wf_t = nc.dram_tensor("Wf", [1, D], F32, kind="ExternalInput")
    b1_t = nc.dram_tensor("b1", [1, D], F32, kind="ExternalInput")
    b2_t = nc.dram_tensor("b2", [1, D], F32, kind="ExternalInput")
    bf_t = nc.dram_tensor("bf", [1, 1], F32, kind="ExternalInput")
    idx_t = nc.dram_tensor("idx", [P, idx_cols], I16, kind="ExternalInput")
    dinv_t = nc.dram_tensor("dinv_rows", [P, R], F32, kind="ExternalInput")
    out_t = nc.dram_tensor("out", [P, R], F32, kind="ExternalOutput")

    rg = [list(range(n_cores))]
    # node-block groups between AllGather boundaries: quarters of 25 blocks
    qblocks = cap // P                  # 25
    groups = []
    for q in range(4):
        b0 = q * qblocks
        full, remn = divmod(qblocks, 4)
        gb = b0
        for _ in range(full):
            groups.append((gb, 4, q, False))
            gb += 4
        if remn:
            groups.append((gb, remn, q, False))
            gb += remn
        groups[-1] = (groups[-1][0], groups[-1][1], q, True)  # AG after last

    with tile.TileContext(nc) as tc:
        with (
            tc.tile_pool(name="dram", bufs=1, space="DRAM") as dpool,
            tc.tile_pool(name="const", bufs=1) as cpool,
            tc.tile_pool(name="sb", bufs=2) as spool,
            tc.tile_pool(name="ps", bufs=2, space="PSUM") as ppool,
        ):
            nc.gpsimd.load_library(library_config.mlp)

            ag_in = [dpool.tile([rows, D], TAB_DT, name=f"ag_in{i}")
                     for i in range(2)]
            table = [dpool.tile([table_rows, D], TAB_DT, name=f"table{i}")
                     for i in range(2)]
            partials = [dpool.tile([2 * half_rows, D], TAB_DT,
                                   name=f"partials{i}")
                        for i in range(2)]

            ident = cpool.tile([P, P], BF16)
            make_identity(nc, ident[:])
            w1s = cpool.tile([D, D], F32)
            nc.sync.dma_start(out=w1s[:], in_=w1_t.ap())
            w2s = cpool.tile([D, D], F32)
            nc.sync.dma_start(out=w2s[:], in_=w2_t.ap())
            w2b = cpool.tile([D, D], BF16)
            nc.vector.tensor_copy(w2b[:], w2s[:])
            wfs = cpool.tile([1, D], F32)
            nc.sync.dma_start(out=wfs[:], in_=wf_t.ap())
            b1s = cpool.tile([1, D], F32)
            nc.sync.dma_start(out=b1s[:], in_=b1_t.ap())
            b2s = cpool.tile([1, D], F32)
            nc.sync.dma_start(out=b2s[:], in_=b2_t.ap())
            bfs = cpool.tile([1, 1], F32)
            nc.sync.dma_start(out=bfs[:], in_=bf_t.ap())
            idx_s = cpool.tile([P, idx_cols], I16)
            nc.sync.dma_start(out=idx_s[:], in_=idx_t.ap())
            dinv_s = cpool.tile([P, R], F32)
            nc.sync.dma_start(out=dinv_s[:], in_=dinv_t.ap())
            dinv_b = cpool.tile([P, R], BF16)
            nc.vector.tensor_copy(dinv_b[:], dinv_s[:])

            ones1 = cpool.tile([1, P], F32)
            nc.vector.memset(ones1[:], 1.0)

            def bcast(vec_ap, n, nm, dt):
                pb = ppool.tile([P, n], F32, tag="pbc")
                nc.tensor.matmul(pb[:], lhsT=ones1[:], rhs=vec_ap, start=True,
                                 stop=True)
                sb = cpool.tile([P, n], dt, name=f"bc_{nm}")
                nc.vector.tensor_copy(sb[:], pb[:])
                return sb

            b1b = bcast(b1s[:], D, "b1", BF16)
            b2b = bcast(b2s[:], D, "b2", BF16)
            wfb = bcast(wfs[:], D, "wf", BF16)
            bfb = bcast(bfs[:], 1, "bf", F32)

            out_stage = cpool.tile([P, R], F32)
            relu1_t = cpool.tile([P, rows], BF16)   # feature-major relu(l1)

            qrr = [0]

            def next_q():
                qrr[0] = (qrr[0] + 1) % 4
                return qrr[0]

            def mm_stage(layer, lhs_dram, lhs_sb, w_tile, lhs_dt):
                """table rows = dinv * (x @ W), 4 node-blocks per group."""
                for (b0, g, q, do_ag) in groups:
                    if lhs_dram is not None:
                        xt = spool.tile([P, 4 * P], lhs_dt, tag="xt")
                        nc.sync.dma_start(
                            out=xt[:, :g * P],
                            in_=lhs_dram.ap()[:, b0 * P:(b0 + g) * P])
                        lsrc = xt
                        loff = 0
                    else:
                        lsrc = lhs_sb
                        loff = b0 * P
                    pm = ppool.tile([P, 4 * D], F32, tag="psm")
                    for i in range(g):
                        nc.tensor.matmul(
                            pm[:, i * D:(i + 1) * D],
                            lhsT=lsrc[:, loff + i * P:loff + (i + 1) * P],
                            rhs=w_tile[:], start=True, stop=True)
                    hc = spool.tile([P, 4 * D], TAB_DT, tag="hc")
                    nc.vector.tensor_tensor(
                        out=hc[:, :g * D].rearrange("p (g f) -> p g f", f=D),
                        in0=pm[:, :g * D].rearrange("p (g f) -> p g f", f=D),
                        in1=dinv_s[:, b0:b0 + g, None].to_broadcast([P, g, D]),
                        op=ALU.mult)
                    nc.sync.dma_start(
                        out=ag_in[layer][b0 * P:(b0 + g) * P, :].rearrange(
                            "(g p) f -> p g f", p=P),
                        in_=hc[:, :g * D].rearrange("p (g f) -> p g f", f=D))
                    if do_ag:
                        nc.gpsimd.collective_compute(
                            "AllGather", ALU.bypass, replica_groups=rg,
                            ins=[ag_in[layer][q * cap:(q + 1) * cap, :]],
                            outs=[table[layer][q * CH:(q + 1) * CH, :]])

            def aggregate(layer):
                tab = table[layer]
                par = partials[layer]
                for meta in call_meta:
                    if meta[0] != "L1":
                        continue
                    (_, c, segcols, ops, r0, nr0, ioff) = meta
                    msg = spool.tile([P, SEG_BLOCKS * D], TAB_DT, tag="msg",
                                     bufs=1)
                    for a in range(0, segcols, CALL_COLS):
                        b = min(a + CALL_COLS, segcols)
                        nc.gpsimd.dma_gather(
                            msg[:, a * D:b * D].rearrange(
                                "p (g f) -> p g f", f=D),
                            tab[c * CH:(c + 1) * CH, :],
                            idx_s[:, ioff + 8 * a:ioff + 8 * b],
                            (b - a) * P, (b - a) * P, D,
                            queue_num=next_q(),
                        )
                    for (o, i2, nb) in ops:
                        nc.vector.tensor_tensor(
                            out=msg[:, o * D:(o + nb) * D],
                            in0=msg[:, o * D:(o + nb) * D],
                            in1=msg[:, i2 * D:(i2 + nb) * D],
                            op=ALU.add)
                    pbase = (c * R + r0) * P
                    nc.scalar.dma_start(
                        out=par[pbase:pbase + nr0 * P, :].rearrange(
                            "(r p) f -> p r f", p=P),
                        in_=msg[:, :nr0 * D].rearrange(
                            "p (r f) -> p r f", f=D))
                # level 2: canonical regather of each dst's 4 partials
                agg = spool.tile([P, R * D], TAB_DT, tag="agg", bufs=1)
                for h in range(2):
                    for meta in call_meta:
                        if meta[0] != "L2" or meta[1] != h:
                            continue
                        (_, _, gc0, gw, gr0, gnr, ioff) = meta
                        buf = spool.tile([P, GMAX * D], TAB_DT, tag="l2b",
                                         bufs=1)
                        for a in range(0, gw, CALL_COLS):
                            b = min(a + CALL_COLS, gw)
                            nc.gpsimd.dma_gather(
                                buf[:, a * D:b * D].rearrange(
                                    "p (g f) -> p g f", f=D),
                                par[h * half_rows:(h + 1) * half_rows, :],
                                idx_s[:, ioff + 8 * a:ioff + 8 * b],
                                (b - a) * P, (b - a) * P, D,
                                queue_num=next_q(),
                            )
                        v = buf[:, :gw * D].rearrange(
                            "p (r t f) -> p r t f", t=2, f=D)
                        aslice = agg[:, gr0 * D:(gr0 + gnr) * D].rearrange(
                            "p (r f) -> p r f", f=D)
                        if h == 0:
                            nc.vector.tensor_tensor(
                                out=aslice, in0=v[:, :, 0, :],
                                in1=v[:, :, 1, :], op=ALU.add)
                        else:
                            nc.vector.tensor_tensor(
                                out=v[:, :, 0, :], in0=v[:, :, 0, :],
                                in1=v[:, :, 1, :], op=ALU.add)
                            nc.vector.tensor_tensor(
                                out=aslice, in0=aslice, in1=v[:, :, 0, :],
                                op=ALU.add)
                return agg

            def scale_bias_relu(agg, bias_b):
                nc.vector.tensor_tensor(
                    out=agg[:].rearrange("p (r f) -> p r f", f=D),
                    in0=agg[:].rearrange("p (r f) -> p r f", f=D),
                    in1=dinv_b[:, :, None].to_broadcast([P, R, D]),
                    op=ALU.mult)
                nc.vector.tensor_tensor(
                    out=agg[:].rearrange("p (r f) -> p r f", f=D),
                    in0=agg[:].rearrange("p (r f) -> p r f", f=D),
                    in1=bias_b[:, None, :].to_broadcast([P, R, D]),
                    op=ALU.add)
                nc.scalar.activation(agg[:], agg[:], ACTF.Relu)

            for _rep in range(repeat):
                # ===== layer 1 =====
                mm_stage(0, x_t, None, w1s, F32)
                agg1 = aggregate(0)
                scale_bias_relu(agg1, b1b)
                # transpose to feature-major (4 blocks per psum tile)
                for gb in range(0, R, 4):
                    pt = ppool.tile([P, 4 * D], BF16, tag="pst")
                    for i in range(4):
                        c = gb + i
                        nc.tensor.transpose(
                            out=pt[:, i * D:(i + 1) * D],
                            in_=agg1[:, c * D:(c + 1) * D],
                            identity=ident[:])
                    nc.scalar.activation(
                        relu1_t[:, gb * P:(gb + 4) * P], pt[:], ACTF.Copy)
                # ===== layer 2 =====
                mm_stage(1, None, relu1_t, w2b, BF16)
                agg2 = aggregate(1)
                scale_bias_relu(agg2, b2b)

                # ===== head: out = relu2 @ Wf + bf =====
                nc.vector.tensor_tensor(
                    out=agg2[:].rearrange("p (r f) -> p r f", f=D),
                    in0=agg2[:].rearrange("p (r f) -> p r f", f=D),
                    in1=wfb[:, None, :].to_broadcast([P, R, D]),
                    op=ALU.mult)
                nc.vector.tensor_reduce(
                    out=out_stage[:],
                    in_=agg2[:].rearrange("p (r f) -> p r f", f=D),
                    axis=AX.X, op=ALU.add)
                nc.vector.tensor_scalar_add(out_stage[:], out_stage[:],
                                            bfb[:, 0:1])
                nc.sync.dma_start(out=out_t.ap(), in_=out_stage[:])

    nc.compile()
    return nc


_CACHE: dict = {}
_PLAN_CACHE: dict = {}


def _plan_cached(edge_index, n_nodes, n_cores):
    import hashlib
    h = hashlib.sha256(np.ascontiguousarray(edge_index).tobytes()).hexdigest()
    key = (h, n_nodes, n_cores)
    if key not in _PLAN_CACHE:
        _PLAN_CACHE[key] = _plan(edge_index, n_nodes, n_cores)
    return _PLAN_CACHE[key]


def _prepare(x, W1, b1, W2, b2, Wf, bf, edge_index, n_nodes, n_cores):
    plan, idx_all, dinv_rows = _plan_cached(edge_index, n_nodes, n_cores)
    shard = plan["shard"]
    rows = plan["rows"]

    x = np.asarray(x, np.float32).reshape(n_cores, shard, D)
    nol = plan["node_of_local"]
    valid = nol >= 0
    x_in = np.zeros((n_cores, rows, D), np.float32)
    x_in[:, valid] = x[:, nol[valid]]

    common = {
        "W1": np.asarray(W1, np.float32).reshape(D, D),
        "W2": np.asarray(W2, np.float32).reshape(D, D),
        "Wf": np.asarray(Wf, np.float32).reshape(1, D),
        "b1": np.asarray(b1, np.float32).reshape(1, D),
        "b2": np.asarray(b2, np.float32).reshape(1, D),
        "bf": np.asarray(bf, np.float32).reshape(1, 1),
    }
    in_maps = []
    for k in range(n_cores):
        m = dict(common)
        m["x_f"] = np.ascontiguousarray(x_in[k].T)
        m["idx"] = np.ascontiguousarray(idx_all[k])
        m["dinv_rows"] = np.ascontiguousarray(dinv_rows[k])
        in_maps.append(m)
    return plan, in_maps


def _collect(results, plan, n_nodes, n_cores):
    shard = plan["shard"]
    nol = plan["node_of_local"]
    valid = np.where(nol >= 0)[0]
    out = np.empty(n_nodes, np.float32)
    for k in range(n_cores):
        vals = results[k]["out"]            # [P, R]
        out[k * shard + nol[valid]] = vals[valid % P, valid // P]
    return out


def kernel(x, W1, b1, W2, b2, Wf, bf, edge_index, _trace=False, _repeat=1,
           _skip=frozenset()):
    plan, in_maps = _prepare(x, W1, b1, W2, b2, Wf, bf, edge_index,
                             N_NODES, N_CORES)
    key = (_repeat, plan["idx_cols"], tuple(m[0] for m in plan["call_meta"]),
           tuple(m[2] for m in plan["call_meta"]))
    if key not in _CACHE:
        _CACHE[key] = _build_program(plan, N_CORES, repeat=_repeat)
    nc = _CACHE[key]
    res = run_bass_kernel_spmd(
        nc, in_maps, core_ids=list(range(N_CORES)), trace=_trace)
    out = _collect(res.results, plan, N_NODES, N_CORES)
    kernel.last_exec_time_ns = res.exec_time_ns
    kernel.last_results = res
    return out


kernel.last_exec_time_ns = None
kernel.last_results = None


# revision 22
# speedup vs baseline: 4.7359x; 1.2161x over previous
"""GCN (2x GCNConv + linear head) on 8 Trainium2 NeuronCores.

Strategy (graph/data parallel per the node-sharding hint):
  - dst nodes sharded across 8 cores (12500 each); 128x128 weights replicated.
  - Symmetric normalization folded into table rows: table[n] = dinv[n]*(x@W)[n];
    aggregate unweighted (self-loop = just another message), multiply by
    dinv[dst] afterwards.
  - Per layer: matmul the core's table shard (x passed feature-major so chunks
    load directly as matmul lhsT; 4 node-blocks batched per PSUM tile) ->
    quarter-wise AllGather of the [102400,128] bf16 table -> bulk gather of
    per-edge messages with `dma_gather` (<=1024 int16 indices per call) ->
    LEVEL-STRIP segmented reduction: within each table chunk, dsts are
    degree-sorted; the grid stores each dst's j-th message in "strip" j
    (one contiguous block range per level), so the whole reduction is one
    strided tensor_tensor add per level pair (pairwise tree over strips)
    instead of per-region trees. Level-1 partials land in DRAM in the same
    (c*R + r)*128 + p row layout; level 2 regathers each dst's 4 partials in
    canonical order (two int16-addressable halves) and combines.
  - This backend charges a large fixed cost per *instruction*, so everything
    is batched into as few, as large instructions as possible; bf16
    throughout the aggregation (output rel-err ~5e-3, gate 2e-2).
  - Known HW pitfalls honored: dma_gather <=1024 idx/call; DVE tensor_tensor
    input dtypes must match (bf16+bf16->f32 output upconvert OK); no
    fine-grained transposing DMAs (PE transposes instead).
"""

import numpy as np

import concourse.bacc as bacc
import concourse.bass as bass
import concourse.mybir as mybir
import concourse.tile as tile
from concourse import library_config
from concourse.bass_utils import run_bass_kernel_spmd
from concourse.masks import make_identity

N_NODES = 100000
N_CORES = 8
D = 128
P = 128

F32 = mybir.dt.float32
BF16 = mybir.dt.bfloat16
I16 = mybir.dt.int16
AX = mybir.AxisListType
ALU = mybir.AluOpType
ACTF = mybir.ActivationFunctionType

TAB_DT = BF16

SEG_BLOCKS = 256   # max grid columns (128-msg blocks) per L1 segment buffer
GMAX = 100         # max grid columns per L2 granule
CALL_COLS = 8      # grid columns per dma_gather call (8*128 = 1024 indices)


def _pack_idx(arr2d):
    """[cols, 128] int array (position i = col*128+p -> arr2d[col, p]) to the
    dma_gather idx tile layout: wrap by 16, replicate to 128 partitions."""
    flat = arr2d.reshape(-1)
    assert flat.size % 16 == 0
    wrapped = flat.reshape(-1, 16).T.astype(np.int16)  # [16, n/16]
    return np.tile(wrapped, (8, 1))  # [128, n/16]


def _granules(widths, gmax):
    out = []
    r0 = 0
    R = len(widths)
    coloff = np.concatenate([[0], np.cumsum(widths)]).astype(int)
    while r0 < R:
        tot = int(widths[r0])
        r1 = r0 + 1
        while r1 < R and tot + int(widths[r1]) <= gmax:
            tot += int(widths[r1])
            r1 += 1
        out.append((r0, r1 - r0, int(coloff[r0]), tot))
        r0 = r1
    return out


def _plan(edge_index, n_nodes, n_cores):
    src = edge_index[0].astype(np.int64)
    dst = edge_index[1].astype(np.int64)
    shard = n_nodes // n_cores
    R = 4 * (-(-shard // (4 * P)))
    rows = R * P
    cap = rows // 4                     # local rows per quarter
    CH = 2 * rows                       # chunk rows = 8 cores * cap
    n_chunks = 4
    assert CH - 1 <= 32767

    base, rem = divmod(shard, 4)
    qreal = np.array([base + (1 if q < rem else 0) for q in range(4)])
    assert (qreal < cap).all()
    qstart = np.concatenate([[0], np.cumsum(qreal)])

    il = np.arange(shard)
    q_of_il = np.searchsorted(qstart, il, "right") - 1
    local_of_il = q_of_il * cap + il - qstart[q_of_il]
    node_of_local = np.full(rows, -1, np.int64)
    node_of_local[local_of_il] = il
    core_of = np.arange(n_nodes) // shard
    loc = local_of_il[np.arange(n_nodes) - core_of * shard]
    pos = (loc // cap) * CH + core_of * cap + loc % cap

    deg = np.bincount(dst, minlength=n_nodes).astype(np.int64) + 1
    dinv = (1.0 / np.sqrt(deg.astype(np.float64))).astype(np.float32)

    MD = np.concatenate([dst, np.arange(n_nodes)])
    MS = np.concatenate([pos[src], pos])
    MC = MS // CH
    NM = MD.shape[0]
    dloc = loc[MD - core_of[MD] * shard]
    dcore = core_of[MD]

    cnt = np.zeros((n_cores, rows, n_chunks), np.int64)
    np.add.at(cnt, (dcore, dloc, MC), 1)

    # per-chunk degree-sorted ranks; region widths shared across cores
    rank = np.zeros((n_cores, rows, n_chunks), np.int64)
    W = np.zeros((n_chunks, R), np.int64)
    for c in range(n_chunks):
        for k in range(n_cores):
            order = np.argsort(-cnt[k, :, c], kind="stable")
            rank[k, order, c] = np.arange(rows)
        sorted_cnt = -np.sort(-cnt[:, :, c], axis=1)
        W[c] = np.maximum(sorted_cnt[:, ::P].max(axis=0), 1)
    assert W.max() <= SEG_BLOCKS

    # ---- L1 level-strip segments (layout shared across cores) ----
    maxW = int(W.max())
    col_of = np.full((n_chunks, R, maxW), -1, np.int64)
    seg_meta = [[] for _ in range(n_chunks)]
    chunk_cols = np.zeros(n_chunks + 1, np.int64)
    for c in range(n_chunks):
        Wc = W[c]
        cum = np.concatenate([[0], np.cumsum(Wc)])
        r0 = 0
        ccol = 0
        while r0 < R:
            r1 = r0 + 1
            while r1 < R and cum[r1 + 1] - cum[r0] <= SEG_BLOCKS:
                r1 += 1
            L = int(Wc[r0])
            posj = np.array([(Wc > j).sum() for j in range(L)])
            Rj = np.clip(np.minimum(posj, r1) - r0, 0, r1 - r0).astype(int)
            offj = np.concatenate([[0], np.cumsum(Rj)]).astype(int)
            segcols = int(offj[-1])
            assert segcols == int(cum[r1] - cum[r0])
            for j in range(L):
                rr = np.arange(r0, r0 + Rj[j])
                col_of[c, rr, j] = ccol + offj[j] + (rr - r0)
            ops = []
            offs = list(offj[:L])
            lens = list(Rj)
            Lv = L
            while Lv > 1:
                h = (Lv + 1) // 2
                for i in range(Lv - h):
                    ops.append((offs[i], offs[i + h], lens[i + h]))
                Lv = h
            seg_meta[c].append((r0, r1 - r0, segcols, ccol, tuple(ops)))
            ccol += segcols
            r0 = r1
        chunk_cols[c + 1] = ccol
    grid_col0 = np.concatenate([[0], np.cumsum(chunk_cols[1:])])
    tot_cols = int(grid_col0[-1])

    # fill grids [n_cores, tot_cols, 128]; pad -> zero row qreal[c]
    grid = np.empty((n_cores, tot_cols, P), np.int16)
    for c in range(n_chunks):
        grid[:, grid_col0[c]:grid_col0[c + 1]] = qreal[c]
    rk = rank[dcore, dloc, MC]
    r_m = rk // P
    p_m = rk % P
    key = (dcore * rows + dloc) * n_chunks + MC
    morder = np.argsort(key, kind="stable")
    ks = key[morder]
    seg_first = np.ones(NM, bool)
    seg_first[1:] = ks[1:] != ks[:-1]
    seg_start_pos = np.where(seg_first)[0]
    seg_id = np.cumsum(seg_first) - 1
    j_sorted = np.arange(NM) - seg_start_pos[seg_id]
    j = np.empty(NM, np.int64)
    j[morder] = j_sorted
    col = grid_col0[MC] + col_of[MC, r_m, j]
    assert (col_of[MC, r_m, j] >= 0).all()
    grid[dcore, col, p_m] = (MS - MC * CH).astype(np.int16)

    # level-1 partial row of (core, local, chunk): (c*R + r)*128 + p
    prow = (rank // P + np.arange(n_chunks)[None, None, :] * R) * P + rank % P

    half_rows = 2 * R * P
    l2 = np.empty((n_cores, 2, 2 * R, P), np.int16)
    s_all = np.arange(rows)
    p2 = s_all % P
    r2 = s_all // P
    for h in range(2):
        for t in range(2):
            c = 2 * h + t
            v = prow[:, :, c] - h * half_rows
            assert v.min() >= 0 and v.max() < half_rows
            l2[:, h, 2 * r2 + t, p2] = v.astype(np.int16)

    call_meta = []
    off = 0

    def emit_l1(c):
        nonlocal off
        for (r0, nr0, segcols, ccol, ops) in seg_meta[c]:
            call_meta.append(("L1", c, segcols, ops, r0, nr0, off))
            off += 8 * segcols

    def emit_l2(h):
        nonlocal off
        for (gr0, gnr, gc0, gw) in _granules([2] * R, GMAX):
            call_meta.append(("L2", h, gc0, gw, gr0, gnr, off))
            off += 8 * gw

    emit_l1(0)
    emit_l1(1)
    emit_l2(0)
    emit_l1(2)
    emit_l1(3)
    emit_l2(1)
    idx_cols = off

    idx_all = np.empty((n_cores, P, idx_cols), np.int16)
    g01 = grid_col0[2]
    for k in range(n_cores):
        idx_all[k] = np.concatenate(
            [_pack_idx(grid[k, :g01]), _pack_idx(l2[k, 0]),
             _pack_idx(grid[k, g01:]), _pack_idx(l2[k, 1])],
            axis=1)

    dinv_rows = np.zeros((n_cores, P, R), np.float32)
    valid = node_of_local >= 0
    lv = np.where(valid)[0]
    for k in range(n_cores):
        dinv_rows[k, lv % P, lv // P] = dinv[k * shard + node_of_local[lv]]

    plan = dict(shard=shard, R=R, rows=rows, CH=CH, cap=cap,
                n_chunks=n_chunks, table_rows=rows * n_cores,
                half_rows=half_rows, call_meta=call_meta, idx_cols=idx_cols,
                tot_cols=tot_cols, node_of_local=node_of_local)
    return plan, idx_all, dinv_rows


def _build_program(plan, n_cores, repeat=1, skip=frozenset()):
    R = plan["R"]
    rows = plan["rows"]
    CH = plan["CH"]
    table_rows = plan["table_rows"]
    half_rows = plan["half_rows"]
    call_meta = plan["call_meta"]
    idx_cols = plan["idx_cols"]
    cap = plan["cap"]

    nc = bacc.Bacc("TRN2", target_bir_lowering=False, debug=False,
                   enable_asserts=False, num_devices=n_cores,
                   num_swdge_queues=4)

    x_t = nc.dram_tensor("x_f", [D, rows], F32, kind="ExternalInput")
    w1_t = nc.dram_tensor("W1", [D, D], F32, kind="ExternalInput")
    w2_t = nc.dram_tensor("W2", [D, D], F32, kind="ExternalInput")
    wf_t = nc.dram_tensor("Wf", [1, D], F32, kind="ExternalInput")
    b1_t = nc.dram_tensor("b1", [1, D], F32, kind="ExternalInput")
    b2_t = nc.dram_tensor("b2", [1, D], F32, kind="ExternalInput")
    bf_t = nc.dram_tensor("bf", [1, 1], F32, kind="ExternalInput")
    idx_t = nc.dram_tensor("idx", [P, idx_cols], I16, kind="ExternalInput")
    dinv_t = nc.dram_tensor("dinv_rows", [P, R], F32, kind="ExternalInput")
    out_t = nc.dram_tensor("out", [P, R], F32, kind="ExternalOutput")

    rg = [list(range(n_cores))]
    # node-block groups between AllGather boundaries: quarters of 25 blocks
    qblocks = cap // P                  # 25
    MMG = 8                             # node-blocks per PSUM group
    groups = []
    for q in range(4):
        b0 = q * qblocks
        full, remn = divmod(qblocks, MMG)
        gb = b0
        for _ in range(full):
            groups.append((gb, MMG, q, False))
            gb += MMG
        if remn:
            groups.append((gb, remn, q, False))
            gb += remn
        groups[-1] = (groups[-1][0], groups[-1][1], q, True)  # AG after last

    with tile.TileContext(nc) as tc:
        with (
            tc.tile_pool(name="dram", bufs=1, space="DRAM") as dpool,
            tc.tile_pool(name="const", bufs=1) as cpool,
            tc.tile_pool(name="sb", bufs=2) as spool,
            tc.tile_pool(name="ps", bufs=2, space="PSUM") as ppool,
        ):
            nc.gpsimd.load_library(library_config.mlp)

            ag_in = [dpool.tile([rows, D], TAB_DT, name=f"ag_in{i}")
                     for i in range(2)]
            table = [dpool.tile([table_rows, D], TAB_DT, name=f"table{i}")
                     for i in range(2)]
            partials = [dpool.tile([2 * half_rows, D], TAB_DT,
                                   name=f"partials{i}")
                        for i in range(2)]

            ident = cpool.tile([P, P], BF16)
            make_identity(nc, ident[:])
            w1s = cpool.tile([D, D], F32)
            nc.sync.dma_start(out=w1s[:], in_=w1_t.ap())
            w2s = cpool.tile([D, D], F32)
            nc.sync.dma_start(out=w2s[:], in_=w2_t.ap())
            w2b = cpool.tile([D, D], BF16)
            nc.vector.tensor_copy(w2b[:], w2s[:])
            wfs = cpool.tile([1, D], F32)
            nc.sync.dma_start(out=wfs[:], in_=wf_t.ap())
            b1s = cpool.tile([1, D], F32)
            nc.sync.dma_start(out=b1s[:], in_=b1_t.ap())
            b2s = cpool.tile([1, D], F32)
            nc.sync.dma_start(out=b2s[:], in_=b2_t.ap())
            bfs = cpool.tile([1, 1], F32)
            nc.sync.dma_start(out=bfs[:], in_=bf_t.ap())
            idx_s = cpool.tile([P, idx_cols], I16)
            nc.sync.dma_start(out=idx_s[:], in_=idx_t.ap())
            dinv_s = cpool.tile([P, R], F32)
            nc.sync.dma_start(out=dinv_s[:], in_=dinv_t.ap())
            dinv_b = cpool.tile([P, R], BF16)
            nc.vector.tensor_copy(dinv_b[:], dinv_s[:])

            ones1 = cpool.tile([1, P], F32)
            nc.vector.memset(ones1[:], 1.0)

            def bcast(vec_ap, n, nm, dt):
                pb = ppool.tile([P, n], F32, tag="pbc")
                nc.tensor.matmul(pb[:], lhsT=ones1[:], rhs=vec_ap, start=True,
                                 stop=True)
                sb = cpool.tile([P, n], dt, name=f"bc_{nm}")
                nc.vector.tensor_copy(sb[:], pb[:])
                return sb

            b1b = bcast(b1s[:], D, "b1", BF16)
            b2b = bcast(b2s[:], D, "b2", BF16)
            wfb = bcast(wfs[:], D, "wf", BF16)
            bfb = bcast(bfs[:], 1, "bf", F32)

            out_stage = cpool.tile([P, R], F32)
            relu1_t = cpool.tile([P, rows], BF16)   # feature-major relu(l1)

            qrr = [0]

            def next_q():
                qrr[0] = (qrr[0] + 1) % 4
                return qrr[0]

            def mm_stage(layer, lhs_dram, lhs_sb, w_tile, lhs_dt):
                """table rows = dinv * (x @ W), 4 node-blocks per group."""
                for (b0, g, q, do_ag) in groups:
                    if lhs_dram is not None:
                        xt = spool.tile([P, 8 * P], lhs_dt, tag="xt")
                        nc.sync.dma_start(
                            out=xt[:, :g * P],
                            in_=lhs_dram.ap()[:, b0 * P:(b0 + g) * P])
                        lsrc = xt
                        loff = 0
                    else:
                        lsrc = lhs_sb
                        loff = b0 * P
                    pm = ppool.tile([P, 8 * D], F32, tag="psm")
                    for i in range(g):
                        nc.tensor.matmul(
                            pm[:, i * D:(i + 1) * D],
                            lhsT=lsrc[:, loff + i * P:loff + (i + 1) * P],
                            rhs=w_tile[:], start=True, stop=True)
                    hc = spool.tile([P, 8 * D], TAB_DT, tag="hc")
                    nc.vector.tensor_tensor(
                        out=hc[:, :g * D].rearrange("p (g f) -> p g f", f=D),
                        in0=pm[:, :g * D].rearrange("p (g f) -> p g f", f=D),
                        in1=dinv_s[:, b0:b0 + g, None].to_broadcast([P, g, D]),
                        op=ALU.mult)
                    nc.sync.dma_start(
                        out=ag_in[layer][b0 * P:(b0 + g) * P, :].rearrange(
                            "(g p) f -> p g f", p=P),
                        in_=hc[:, :g * D].rearrange("p (g f) -> p g f", f=D))
                    if do_ag:
                        nc.gpsimd.collective_compute(
                            "AllGather", ALU.bypass, replica_groups=rg,
                            ins=[ag_in[layer][q * cap:(q + 1) * cap, :]],
                            outs=[table[layer][q * CH:(q + 1) * CH, :]])

            def aggregate(layer):
                tab = table[layer]
                par = partials[layer]
                for meta in call_meta:
                    if meta[0] != "L1":
                        continue
                    (_, c, segcols, ops, r0, nr0, ioff) = meta
                    msg = spool.tile([P, SEG_BLOCKS * D], TAB_DT, tag="msg",
                                     bufs=1)
                    for a in range(0, segcols, CALL_COLS):
                        b = min(a + CALL_COLS, segcols)
                        nc.gpsimd.dma_gather(
                            msg[:, a * D:b * D].rearrange(
                                "p (g f) -> p g f", f=D),
                            tab[c * CH:(c + 1) * CH, :],
                            idx_s[:, ioff + 8 * a:ioff + 8 * b],
                            (b - a) * P, (b - a) * P, D,
                            queue_num=next_q(),
                        )
                    for (o, i2, nb) in ops:
                        nc.vector.tensor_tensor(
                            out=msg[:, o * D:(o + nb) * D],
                            in0=msg[:, o * D:(o + nb) * D],
                            in1=msg[:, i2 * D:(i2 + nb) * D],
                            op=ALU.add)
                    pbase = (c * R + r0) * P
                    nc.scalar.dma_start(
                        out=par[pbase:pbase + nr0 * P, :].rearrange(
                            "(r p) f -> p r f", p=P),
                        in_=msg[:, :nr0 * D].rearrange(
                            "p (r f) -> p r f", f=D))
                # level 2: canonical regather of each dst's 4 partials
                agg = spool.tile([P, R * D], TAB_DT, tag="agg", bufs=1)
                for h in range(2):
                    for meta in call_meta:
                        if meta[0] != "L2" or meta[1] != h:
                            continue
                        (_, _, gc0, gw, gr0, gnr, ioff) = meta
                        buf = spool.tile([P, GMAX * D], TAB_DT, tag="l2b",
                                         bufs=1)
                        for a in range(0, gw, CALL_COLS):
                            b = min(a + CALL_COLS, gw)
                            nc.gpsimd.dma_gather(
                                buf[:, a * D:b * D].rearrange(
                                    "p (g f) -> p g f", f=D),
                                par[h * half_rows:(h + 1) * half_rows, :],
                                idx_s[:, ioff + 8 * a:ioff + 8 * b],
                                (b - a) * P, (b - a) * P, D,
                                queue_num=next_q(),
                            )
                        v = buf[:, :gw * D].rearrange(
                            "p (r t f) -> p r t f", t=2, f=D)
                        aslice = agg[:, gr0 * D:(gr0 + gnr) * D].rearrange(
                            "p (r f) -> p r f", f=D)
                        if h == 0:
                            nc.vector.tensor_tensor(
                                out=aslice, in0=v[:, :, 0, :],
                                in1=v[:, :, 1, :], op=ALU.add)
                        else:
                            nc.vector.tensor_tensor(
                                out=v[:, :, 0, :], in0=v[:, :, 0, :],
                                in1=v[:, :, 1, :], op=ALU.add)
                            nc.vector.tensor_tensor(
                                out=aslice, in0=aslice, in1=v[:, :, 0, :],
                                op=ALU.add)
                return agg

            def scale_bias_relu(agg, bias_b):
                nc.vector.tensor_tensor(
                    out=agg[:].rearrange("p (r f) -> p r f", f=D),
                    in0=agg[:].rearrange("p (r f) -> p r f", f=D),
                    in1=dinv_b[:, :, None].to_broadcast([P, R, D]),
                    op=ALU.mult)
                nc.vector.tensor_tensor(
                    out=agg[:].rearrange("p (r f) -> p r f", f=D),
                    in0=agg[:].rearrange("p (r f) -> p r f", f=D),
                    in1=bias_b[:, None, :].to_broadcast([P, R, D]),
                    op=ALU.add)
                nc.scalar.activation(agg[:], agg[:], ACTF.Relu)

            for _rep in range(repeat):
                # ===== layer 1 =====
                mm_stage(0, x_t, None, w1s, F32)
                agg1 = aggregate(0)
                scale_bias_relu(agg1, b1b)
                # transpose to feature-major (8 blocks per psum tile)
                for gb in range(0, R, 8):
                    g = min(8, R - gb)
                    pt = ppool.tile([P, 8 * D], BF16, tag="pst")
                    for i in range(g):
                        c = gb + i
                        nc.tensor.transpose(
                            out=pt[:, i * D:(i + 1) * D],
                            in_=agg1[:, c * D:(c + 1) * D],
                            identity=ident[:])
                    nc.scalar.activation(
                        relu1_t[:, gb * P:(gb + g) * P], pt[:, :g * D],
                        ACTF.Copy)
                # ===== layer 2 =====
                mm_stage(1, None, relu1_t, w2b, BF16)
                agg2 = aggregate(1)
                scale_bias_relu(agg2, b2b)

                # ===== head: out = relu2 @ Wf + bf =====
                nc.vector.tensor_tensor(
                    out=agg2[:].rearrange("p (r f) -> p r f", f=D),
                    in0=agg2[:].rearrange("p (r f) -> p r f", f=D),
                    in1=wfb[:, None, :].to_broadcast([P, R, D]),
                    op=ALU.mult)
                nc.vector.tensor_reduce(
                    out=out_stage[:],
                    in_=agg2[:].rearrange("p (r f) -> p r f", f=D),
                    axis=AX.X, op=ALU.add)
                nc.vector.tensor_scalar_add(out_stage[:], out_stage[:],
                                            bfb[:, 0:1])
                nc.sync.dma_start(out=out_t.ap(), in_=out_stage[:])

    nc.compile()
    return nc


_CACHE: dict = {}
_PLAN_CACHE: dict = {}


def _plan_cached(edge_index, n_nodes, n_cores):
    import hashlib
    h = hashlib.sha256(np.ascontiguousarray(edge_index).tobytes()).hexdigest()
    key = (h, n_nodes, n_cores)
    if key not in _PLAN_CACHE:
        _PLAN_CACHE[key] = _plan(edge_index, n_nodes, n_cores)
    return _PLAN_CACHE[key]


def _prepare(x, W1, b1, W2, b2, Wf, bf, edge_index, n_nodes, n_cores):
    plan, idx_all, dinv_rows = _plan_cached(edge_index, n_nodes, n_cores)
    shard = plan["shard"]
    rows = plan["rows"]

    x = np.asarray(x, np.float32).reshape(n_cores, shard, D)
    nol = plan["node_of_local"]
    valid = nol >= 0
    x_in = np.zeros((n_cores, rows, D), np.float32)
    x_in[:, valid] = x[:, nol[valid]]

    common = {
        "W1": np.asarray(W1, np.float32).reshape(D, D),
        "W2": np.asarray(W2, np.float32).reshape(D, D),
        "Wf": np.asarray(Wf, np.float32).reshape(1, D),
        "b1": np.asarray(b1, np.float32).reshape(1, D),
        "b2": np.asarray(b2, np.float32).reshape(1, D),
        "bf": np.asarray(bf, np.float32).reshape(1, 1),
    }
    in_maps = []
    for k in range(n_cores):
        m = dict(common)
        m["x_f"] = np.ascontiguousarray(x_in[k].T)
        m["idx"] = np.ascontiguousarray(idx_all[k])
        m["dinv_rows"] = np.ascontiguousarray(dinv_rows[k])
        in_maps.append(m)
    return plan, in_maps


def _collect(results, plan, n_nodes, n_cores):
    shard = plan["shard"]
    nol = plan["node_of_local"]
    valid = np.where(nol >= 0)[0]
    out = np.empty(n_nodes, np.float32)
    for k in range(n_cores):
        vals = results[k]["out"]            # [P, R]
        out[k * shard + nol[valid]] = vals[valid % P, valid // P]
    return out


def kernel(x, W1, b1, W2, b2, Wf, bf, edge_index, _trace=False, _repeat=1,
           _skip=frozenset()):
    plan, in_maps = _prepare(x, W1, b1, W2, b2, Wf, bf, edge_index,
                             N_NODES, N_CORES)
    key = (_repeat, plan["idx_cols"], tuple(m[0] for m in plan["call_meta"]),
           tuple(m[2] for m in plan["call_meta"]))
    if key not in _CACHE:
        _CACHE[key] = _build_program(plan, N_CORES, repeat=_repeat)
    nc = _CACHE[key]
    res = run_bass_kernel_spmd(
        nc, in_maps, core_ids=list(range(N_CORES)), trace=_trace)
    out = _collect(res.results, plan, N_NODES, N_CORES)
    kernel.last_exec_time_ns = res.exec_time_ns
    kernel.last_results = res
    return out


kernel.last_exec_time_ns = None
kernel.last_results = None
